# revision 45
# baseline (speedup 1.0000x reference)
"""AdaptiveAdjacency Bass kernel for 8 TRN2 NeuronCores.

Reference computation per batch b (N=1024 nodes, H=24 hidden):
    Z   = relu(xt @ W + b)                    (N, H)
    A   = Z @ Z.T                             (N, N)  -- symmetric!
    A   = 0.5*(softmax(A, -1) + softmax(A, -2)) + I
    deg = A.sum(-1);  out = A * deg^-1/2 [row] * deg^-1/2 [col]

Math used here (exploiting symmetry of A_raw):
    E = exp(A_raw - 40)            (shift is softmax-invariant; A_raw max ~54)
    softmax(A,-2) == softmax(A,-1).T, so with r = 1/rowsum(E):
        A_sym[n,m] = E[n,m] * 0.5*(r[n]+r[m]) + I
    Fold "+I" into E:  E' = E + diag(rowsum)  =>  out = E' * C with
        C[n,m] = u[n]v[m] + v[n]u[m],  u = 0.5*r*ds,  v = ds,
        ds = degree^-1/2, degree = 1 + 0.5*colsum(r[n]*E'[n,m])
    C is rank-2 -> one K=2 matmul per output tile; single elementwise
    multiply per output element (the only full-size DVE pass).

Sharding: data-parallel over B=32 across 8 cores (4 batches each);
W/b replicated. Host pre-transposes xt to (B, F, N) fp16 so the tiny
Linear runs as a natural PE matmul (contraction over F on partitions).
"""

import numpy as np

import concourse.bass as bass
import concourse.tile as tile
from concourse import bacc, mybir
from concourse.masks import make_identity
from concourse.bass_utils import run_bass_kernel_spmd

B_FULL = 32
B_LOC = 4  # batches per core
N = 1024
F = 64
H = 24
NT = N // 128  # 8 row tiles
CK = 512  # matmul free chunk (one PSUM bank)
NCK = N // CK
KSHIFT = -40.0  # softmax shift (global constant: softmax-invariant)
N_CORES = 8

f32 = mybir.dt.float32
bf16 = mybir.dt.bfloat16
fp16 = mybir.dt.float16
AF = mybir.ActivationFunctionType
ALU = mybir.AluOpType


_TABLES_PATCHED = False


def _force_single_act_table_set():
    """All activation funcs used here (Exp, Ln, Relu, Copy/Identity) live in
    the natural_log_exp_and_others set. bacc's table-load inserter picks the
    first set containing each function, which thrashes ~2.7us per switch
    between exp_and_others and natural_log. Strip those functions from every
    other set (indices must be preserved) so one table load covers the
    whole kernel."""
    global _TABLES_PATCHED
    if _TABLES_PATCHED:
        return
    _TABLES_PATCHED = True
    import concourse.hw_specs as hw_specs

    orig = hw_specs.get_activation_tables
    keep = {
        AF.Exp,
        AF.Ln,
        AF.Relu,
        AF.Copy,
        AF.Identity,
        AF.Square,
        AF.Abs,
        AF.Sign,
        AF.MemsetZero,
        AF.Is_finite,
    }
    target = "natural_log_exp_and_others"

    def patched(module_arch):
        tables = orig(module_arch)
        if target not in tables:
            return tables
        out = {}
        for name, funcs in tables.items():
            out[name] = funcs if name == target else (funcs - keep)
        return out

    hw_specs.get_activation_tables = patched
    bacc.get_activation_tables = patched


def build_nc(repeat: int = 1) -> bass.Bass:
    """repeat>1 builds a timing variant that executes the whole computation
    `repeat` times (same outputs overwritten) so real device time per
    iteration can be measured by wall-clock differencing."""
    _force_single_act_table_set()
    nc = bacc.Bacc()
    xtT = nc.declare_dram_parameter("xtT", [B_LOC, F, N], fp16, isOutput=False)
    Wd = nc.declare_dram_parameter("W", [F, H], fp16, isOutput=False)
    bd = nc.declare_dram_parameter("b", [H, 1], f32, isOutput=False)
    outd = nc.declare_dram_parameter("out", [B_LOC, N, N], bf16, isOutput=True)
    # host constants (engine APs must start at partition 0, so these cannot
    # be built with sliced memsets):
    #   cst col 0/1: per-partition scale/bias for the fused colsum Ln
    #   cuv cols 0:2 = Cu, 2:4 = Cv (f32r lhsT for the log-mix matmuls)
    cstd = nc.declare_dram_parameter("cst", [2, 2], f32, isOutput=False)
    cuvd = nc.declare_dram_parameter("cuv", [2, 4], mybir.dt.float32r, isOutput=False)

    with tile.TileContext(nc) as tc:
        with (
            tc.tile_pool(name="singles", bufs=1) as singles,
            tc.tile_pool(name="zpool", bufs=B_LOC) as zpool,
            tc.tile_pool(name="epool", bufs=2 + 2 * NT) as epool,
            tc.tile_pool(name="vpool", bufs=2) as vpool,
            tc.tile_pool(name="opool", bufs=4) as opool,
            tc.tile_pool(name="apool", bufs=2, space="PSUM") as apool,
            tc.tile_pool(name="cpool", bufs=2, space="PSUM") as cpool,
            tc.tile_pool(name="cspool", bufs=1, space="PSUM") as cspool,
        ):
            wsb = singles.tile([F, H], fp16)
            nc.gpsimd.dma_start(wsb[:], Wd[:, :])
            bsb = singles.tile([H, 1], f32)
            nc.gpsimd.dma_start(bsb[:], bd[:, :])
            ident = singles.tile([128, 128], bf16)
            make_identity(nc, ident[:])
            cm40 = singles.tile([128, 1], f32)
            nc.gpsimd.memset(cm40[:], KSHIFT)
            # per-partition [scale, bias] for the fused colsum Ln:
            # row 0: ln(0.5*cs0 + 1.0)   row 1: ln(1.0*cs1 + 0.0)
            cstsb = singles.tile([2, 2], f32)
            nc.gpsimd.dma_start(cstsb[:], cstd[:, :])
            cuvsb = singles.tile([2, 4], mybir.dt.float32r)
            nc.gpsimd.dma_start(cuvsb[:], cuvd[:, :])

            # ---- Z^T = relu(W^T @ xt^T + b) : [H, N] fp16, all batches
            # upfront (fills otherwise-idle engines during pipeline fill and
            # removes the Z chain from the batch-boundary critical path) ----
            zts = []
            for b in range(B_LOC):
                xtsb = zpool.tile([F, N], fp16, tag="xt")
                nc.sync.dma_start(xtsb[:], xtT[b])
                zpsum = apool.tile([H, N], f32, tag="ps")
                for j in range(NCK):
                    nc.tensor.matmul(
                        zpsum[:, j * CK : (j + 1) * CK],
                        wsb[:],
                        xtsb[:, j * CK : (j + 1) * CK],
                        start=True,
                        stop=True,
                    )
                zt = zpool.tile([H, N], fp16, tag="zt")
                # relu on DVE (ACT is the bottleneck engine): (Zpre + b) max 0
                nc.vector.tensor_scalar(
                    zt[:], zpsum[:], bsb[:], 0.0, ALU.add, ALU.max
                )
                zts.append(zt)

            def stats_tile(b, i, zt, rowsums, r_buf, cs, e_tiles):
                """A_raw matmul, exp(+rowsum), diag fix, r_i, colsum accum."""
                apsum = apool.tile([128, N], f32, tag="ps")
                for j in range(NCK):
                    nc.tensor.matmul(
                        apsum[:, j * CK : (j + 1) * CK],
                        zt[:, i * 128 : (i + 1) * 128],
                        zt[:, j * CK : (j + 1) * CK],
                        start=True,
                        stop=True,
                    )
                et = epool.tile([128, N], bf16, tag="E")
                nc.scalar.activation(
                    et[:],
                    apsum[:],
                    AF.Exp,
                    bias=cm40[:],
                    accum_out=rowsums[:, i : i + 1],
                )
                # E'[n,n] += rowsum[n] (folds the "+I" into the final multiply)
                nc.vector.scalar_tensor_tensor(
                    out=et[:, i * 128 : (i + 1) * 128],
                    in0=ident[:],
                    scalar=rowsums[:, i : i + 1],
                    in1=et[:, i * 128 : (i + 1) * 128],
                    op0=ALU.mult,
                    op1=ALU.add,
                )
                with nc.allow_low_precision("bf16 r for colsum lhsT"):
                    nc.vector.reciprocal(r_buf[:, i, 0:1], rowsums[:, i : i + 1])
                for j in range(NCK):
                    nc.tensor.matmul(
                        cs[:, j * CK : (j + 1) * CK],
                        r_buf[:, i, :],
                        et[:, j * CK : (j + 1) * CK],
                        start=(i == 0),
                        stop=(i == NT - 1),
                        skip_group_check=True,
                    )
                e_tiles.append(et)

            def batch_tail(b, cs):
                """degree -> ds; u, v vectors (free layout).
                degree = 1 + 0.5*cs0 ; v = ds = exp(-0.5*ln(degree))
                u = 0.5*r*ds = exp(-ln(cs1) - 0.5*ln(degree))  (cs1 = 2*rowsum)
                Engine ops must be lane-aligned (partition base 0), so the
                log-domain row mixing runs on the PE (K=2 f32r matmuls
                against a tiny constant lhsT), never across partitions."""
                uv = vpool.tile([2, N], bf16, tag="uv")  # [u; v] (lhsT source)
                vu = vpool.tile([2, N], bf16, tag="vu")  # [v; u] (rhs source)
                lls = vpool.tile([2, N], mybir.dt.float32r, tag="lls")
                nc.scalar.activation(
                    lls[:], cs[:, :], AF.Ln, bias=cstsb[:, 1:2], scale=cstsb[:, 0:1]
                )
                # log-mix matmuls write cpool slots (idle until the c-phase),
                # so the cs accumulator frees right after the Ln and the next
                # batch's colsum can begin during this batch's tail
                for coeff, dst in ((0, uv), (2, vu)):
                    for j in range(NCK):
                        lmix = cpool.tile([2, CK], f32, tag="c")
                        nc.tensor.matmul(
                            lmix[:],
                            cuvsb[:, coeff : coeff + 2],
                            lls[:, j * CK : (j + 1) * CK],
                            start=True,
                            stop=True,
                        )
                        nc.scalar.activation(
                            dst[:, j * CK : (j + 1) * CK], lmix[:], AF.Exp
                        )
                return uv, vu

            def c_tile(b, i, uv, vu, e_tiles, last_batch):
                """C = u v^T + v u^T (K=2 matmul), out = E' * C, DMA out."""
                osb = opool.tile([128, N], bf16, tag="o")
                for j in range(NCK):
                    cps = cpool.tile([128, CK], f32, tag="c")
                    nc.tensor.matmul(
                        cps[:],
                        uv[:, i * 128 : (i + 1) * 128],
                        vu[:, j * CK : (j + 1) * CK],
                        start=True,
                        stop=True,
                    )
                    if last_batch and j % 2 == 1:
                        # drain phase: ACT and Pool are idle, DVE is the
                        # bottleneck -- route half the final multiplies
                        # through a PSUM->SBUF copy (DVE 2x) + Pool multiply
                        csb = opool.tile([128, CK], bf16, tag="csb")
                        nc.vector.tensor_copy(csb[:], cps[:])
                        nc.gpsimd.tensor_tensor(
                            osb[:, j * CK : (j + 1) * CK],
                            e_tiles[i][:, j * CK : (j + 1) * CK],
                            csb[:],
                            ALU.mult,
                        )
                    else:
                        nc.vector.tensor_tensor(
                            osb[:, j * CK : (j + 1) * CK],
                            e_tiles[i][:, j * CK : (j + 1) * CK],
                            cps[:],
                            ALU.mult,
                        )
                nc.sync.dma_start(outd[b, i * 128 : (i + 1) * 128, :], osb[:])

            # software pipeline: batch b's stats tiles interleave with batch
            # b-1's output tiles so PE/DVE/DMA trail ACT by one phase
            for rep in range(repeat):
                prev = None
                for b in range(B_LOC):
                    rowsums = vpool.tile([128, NT], f32, tag="rowsums")
                    r_buf = vpool.tile([128, NT, 2], bf16, tag="rbuf")
                    nc.gpsimd.memset(r_buf[:], 1.0)
                    # cs[0,m] = sum_n r[n] E'[n,m] = w'; cs[1,m] = 2*rowsum[m]
                    cs = cspool.tile([2, N], f32, tag="cs")
                    e_tiles = []
                    for i in range(NT):
                        stats_tile(b, i, zts[b], rowsums, r_buf, cs, e_tiles)
                        if prev is not None:
                            c_tile(prev[0], i, prev[1], prev[2], prev[3], False)
                    uv, vu = batch_tail(b, cs)
                    prev = (b, uv, vu, e_tiles)
                last = rep == repeat - 1
                for i in range(NT):
                    c_tile(prev[0], i, prev[1], prev[2], prev[3], last)

    nc.finalize()
    return nc


_NC_CACHE = None


def _get_nc() -> bass.Bass:
    global _NC_CACHE
    if _NC_CACHE is None:
        _NC_CACHE = build_nc()
    return _NC_CACHE


def _make_in_maps(xt: np.ndarray, W: np.ndarray, b: np.ndarray):
    xtT = np.ascontiguousarray(np.asarray(xt).transpose(0, 2, 1)).astype(np.float16)
    Wh = np.ascontiguousarray(np.asarray(W)).astype(np.float16)
    bh = np.ascontiguousarray(np.asarray(b)).reshape(H, 1).astype(np.float32)
    # cst cols: [Ln scale, Ln bias]; cuv cols: Cu(2x2), Cv(2x2)
    # Cu/Cv columns build [ln u; ln v] / [ln v; ln u] from [ln deg; ln(2 rs)]
    # u = exp(-0.5*ldeg - lrs2), v = exp(-0.5*ldeg)
    cst = np.array([[0.5, 1.0], [1.0, 0.0]], dtype=np.float32)
    cuv = np.array(
        [[-0.5, -0.5, -0.5, -0.5], [-1.0, 0.0, 0.0, -1.0]], dtype=np.float32
    )
    return [
        {
            "xtT": xtT[B_LOC * k : B_LOC * (k + 1)],
            "W": Wh,
            "b": bh,
            "cst": cst,
            "cuv": cuv,
        }
        for k in range(N_CORES)
    ]


def run(xt, W, b, trace: bool = False):
    """Run on 8 NeuronCores; returns (out, BassKernelResults)."""
    res = run_bass_kernel_spmd(
        _get_nc(), _make_in_maps(xt, W, b), core_ids=list(range(N_CORES)), trace=trace
    )
    out = np.concatenate(
        [np.asarray(res.results[k]["out"]) for k in range(N_CORES)], axis=0
    )
    return out.astype(np.float32, copy=False), res


def kernel(xt: np.ndarray, W: np.ndarray, b: np.ndarray) -> np.ndarray:
    out, _ = run(xt, W, b, trace=False)
    return out


# revision 47
# speedup vs baseline: 106.2180x; 106.2180x over previous
"""AdaptiveAdjacency Bass kernel for 8 TRN2 NeuronCores.

Reference computation per batch b (N=1024 nodes, H=24 hidden):
    Z   = relu(xt @ W + b)                    (N, H)
    A   = Z @ Z.T                             (N, N)  -- symmetric!
    A   = 0.5*(softmax(A, -1) + softmax(A, -2)) + I
    deg = A.sum(-1);  out = A * deg^-1/2 [row] * deg^-1/2 [col]

Math used here (exploiting symmetry of A_raw):
    E = exp(A_raw - 40)            (shift is softmax-invariant; A_raw max ~54)
    softmax(A,-2) == softmax(A,-1).T, so with r = 1/rowsum(E):
        A_sym[n,m] = E[n,m] * 0.5*(r[n]+r[m]) + I
    Fold "+I" into E:  E' = E + diag(rowsum)  =>  out = E' * C with
        C[n,m] = u[n]v[m] + v[n]u[m],  u = 0.5*r*ds,  v = ds,
        ds = degree^-1/2, degree = 1 + 0.5*colsum(r[n]*E'[n,m])
    C is rank-2 -> one K=2 matmul per output tile; single elementwise
    multiply per output element (the only full-size DVE pass).

Sharding: data-parallel over B=32 across 8 cores (4 batches each);
W/b replicated. Host pre-transposes xt to (B, F, N) fp16 so the tiny
Linear runs as a natural PE matmul (contraction over F on partitions).
"""

import numpy as np

import concourse.bass as bass
import concourse.tile as tile
from concourse import bacc, mybir
from concourse.masks import make_identity
from concourse.bass_utils import run_bass_kernel_spmd

B_FULL = 32
B_LOC = 4  # batches per core
N = 1024
F = 64
H = 24
NT = N // 128  # 8 row tiles
CK = 512  # matmul free chunk (one PSUM bank)
NCK = N // CK
KSHIFT = -40.0  # softmax shift (global constant: softmax-invariant)
N_CORES = 8

f32 = mybir.dt.float32
bf16 = mybir.dt.bfloat16
fp16 = mybir.dt.float16
AF = mybir.ActivationFunctionType
ALU = mybir.AluOpType


_TABLES_PATCHED = False


def _force_single_act_table_set():
    """All activation funcs used here (Exp, Ln, Relu, Copy/Identity) live in
    the natural_log_exp_and_others set. bacc's table-load inserter picks the
    first set containing each function, which thrashes ~2.7us per switch
    between exp_and_others and natural_log. Strip those functions from every
    other set (indices must be preserved) so one table load covers the
    whole kernel."""
    global _TABLES_PATCHED
    if _TABLES_PATCHED:
        return
    _TABLES_PATCHED = True
    import concourse.hw_specs as hw_specs

    orig = hw_specs.get_activation_tables
    keep = {
        AF.Exp,
        AF.Ln,
        AF.Relu,
        AF.Copy,
        AF.Identity,
        AF.Square,
        AF.Abs,
        AF.Sign,
        AF.MemsetZero,
        AF.Is_finite,
    }
    target = "natural_log_exp_and_others"

    def patched(module_arch):
        tables = orig(module_arch)
        if target not in tables:
            return tables
        out = {}
        for name, funcs in tables.items():
            out[name] = funcs if name == target else (funcs - keep)
        return out

    hw_specs.get_activation_tables = patched
    bacc.get_activation_tables = patched


def build_nc(repeat: int = 1, timing_trip: int | None = None) -> bass.Bass:
    """timing_trip=T builds a timing variant: the whole computation runs in
    an on-device For_i loop T times, writing to internal DRAM scratch with a
    tiny external output, so real device time per iteration can be measured
    by wall-clock differencing of two trip counts (fixed host/transfer costs
    cancel; code size is constant)."""
    _force_single_act_table_set()
    nc = bacc.Bacc()
    xtT = nc.declare_dram_parameter("xtT", [B_LOC, F, N], fp16, isOutput=False)
    Wd = nc.declare_dram_parameter("W", [F, H], fp16, isOutput=False)
    bd = nc.declare_dram_parameter("b", [H, 1], f32, isOutput=False)
    if timing_trip is None:
        outd = nc.declare_dram_parameter("out", [B_LOC, N, N], bf16, isOutput=True)
    else:
        outd = nc.dram_tensor("oscratch", [B_LOC, N, N], bf16)
        tiny_out = nc.declare_dram_parameter("out", [2, 2], f32, isOutput=True)
    # host constants (engine APs must start at partition 0, so these cannot
    # be built with sliced memsets):
    #   cst col 0/1: per-partition scale/bias for the fused colsum Ln
    #   cuv cols 0:2 = Cu, 2:4 = Cv (f32r lhsT for the log-mix matmuls)
    cstd = nc.declare_dram_parameter("cst", [2, 2], f32, isOutput=False)
    cuvd = nc.declare_dram_parameter("cuv", [2, 4], mybir.dt.float32r, isOutput=False)

    with tile.TileContext(nc) as tc:
        with (
            tc.tile_pool(name="singles", bufs=1) as singles,
            tc.tile_pool(name="zpool", bufs=B_LOC) as zpool,
            tc.tile_pool(name="epool", bufs=2 + 2 * NT) as epool,
            tc.tile_pool(name="vpool", bufs=2) as vpool,
            tc.tile_pool(name="opool", bufs=4) as opool,
            tc.tile_pool(name="apool", bufs=2, space="PSUM") as apool,
            tc.tile_pool(name="cpool", bufs=2, space="PSUM") as cpool,
            tc.tile_pool(name="cspool", bufs=1, space="PSUM") as cspool,
        ):
            wsb = singles.tile([F, H], fp16)
            nc.gpsimd.dma_start(wsb[:], Wd[:, :])
            bsb = singles.tile([H, 1], f32)
            nc.gpsimd.dma_start(bsb[:], bd[:, :])
            ident = singles.tile([128, 128], bf16)
            make_identity(nc, ident[:])
            cm40 = singles.tile([128, 1], f32)
            nc.gpsimd.memset(cm40[:], KSHIFT)
            # per-partition [scale, bias] for the fused colsum Ln:
            # row 0: ln(0.5*cs0 + 1.0)   row 1: ln(1.0*cs1 + 0.0)
            cstsb = singles.tile([2, 2], f32)
            nc.gpsimd.dma_start(cstsb[:], cstd[:, :])
            cuvsb = singles.tile([2, 4], mybir.dt.float32r)
            nc.gpsimd.dma_start(cuvsb[:], cuvd[:, :])

            # ---- Z^T = relu(W^T @ xt^T + b) : [H, N] fp16, all batches
            # upfront (fills otherwise-idle engines during pipeline fill and
            # removes the Z chain from the batch-boundary critical path) ----
            zts = []
            for b in range(B_LOC):
                xtsb = zpool.tile([F, N], fp16, tag="xt")
                nc.sync.dma_start(xtsb[:], xtT[b])
                zpsum = apool.tile([H, N], f32, tag="ps")
                for j in range(NCK):
                    nc.tensor.matmul(
                        zpsum[:, j * CK : (j + 1) * CK],
                        wsb[:],
                        xtsb[:, j * CK : (j + 1) * CK],
                        start=True,
                        stop=True,
                    )
                zt = zpool.tile([H, N], fp16, tag="zt")
                # relu on DVE (ACT is the bottleneck engine): (Zpre + b) max 0
                nc.vector.tensor_scalar(
                    zt[:], zpsum[:], bsb[:], 0.0, ALU.add, ALU.max
                )
                zts.append(zt)

            def stats_tile(b, i, zt, rowsums, r_buf, cs, e_tiles):
                """A_raw matmul, exp(+rowsum), diag fix, r_i, colsum accum."""
                apsum = apool.tile([128, N], f32, tag="ps")
                for j in range(NCK):
                    nc.tensor.matmul(
                        apsum[:, j * CK : (j + 1) * CK],
                        zt[:, i * 128 : (i + 1) * 128],
                        zt[:, j * CK : (j + 1) * CK],
                        start=True,
                        stop=True,
                    )
                et = epool.tile([128, N], bf16, tag="E")
                nc.scalar.activation(
                    et[:],
                    apsum[:],
                    AF.Exp,
                    bias=cm40[:],
                    accum_out=rowsums[:, i : i + 1],
                )
                # E'[n,n] += rowsum[n] (folds the "+I" into the final multiply)
                nc.vector.scalar_tensor_tensor(
                    out=et[:, i * 128 : (i + 1) * 128],
                    in0=ident[:],
                    scalar=rowsums[:, i : i + 1],
                    in1=et[:, i * 128 : (i + 1) * 128],
                    op0=ALU.mult,
                    op1=ALU.add,
                )
                with nc.allow_low_precision("bf16 r for colsum lhsT"):
                    nc.vector.reciprocal(r_buf[:, i, 0:1], rowsums[:, i : i + 1])
                for j in range(NCK):
                    nc.tensor.matmul(
                        cs[:, j * CK : (j + 1) * CK],
                        r_buf[:, i, :],
                        et[:, j * CK : (j + 1) * CK],
                        start=(i == 0),
                        stop=(i == NT - 1),
                        skip_group_check=True,
                    )
                e_tiles.append(et)

            def batch_tail(b, cs):
                """degree -> ds; u, v vectors (free layout).
                degree = 1 + 0.5*cs0 ; v = ds = exp(-0.5*ln(degree))
                u = 0.5*r*ds = exp(-ln(cs1) - 0.5*ln(degree))  (cs1 = 2*rowsum)
                Engine ops must be lane-aligned (partition base 0), so the
                log-domain row mixing runs on the PE (K=2 f32r matmuls
                against a tiny constant lhsT), never across partitions."""
                uv = vpool.tile([2, N], bf16, tag="uv")  # [u; v] (lhsT source)
                vu = vpool.tile([2, N], bf16, tag="vu")  # [v; u] (rhs source)
                lls = vpool.tile([2, N], mybir.dt.float32r, tag="lls")
                nc.scalar.activation(
                    lls[:], cs[:, :], AF.Ln, bias=cstsb[:, 1:2], scale=cstsb[:, 0:1]
                )
                # log-mix matmuls write cpool slots (idle until the c-phase),
                # so the cs accumulator frees right after the Ln and the next
                # batch's colsum can begin during this batch's tail
                for coeff, dst in ((0, uv), (2, vu)):
                    for j in range(NCK):
                        lmix = cpool.tile([2, CK], f32, tag="c")
                        nc.tensor.matmul(
                            lmix[:],
                            cuvsb[:, coeff : coeff + 2],
                            lls[:, j * CK : (j + 1) * CK],
                            start=True,
                            stop=True,
                        )
                        nc.scalar.activation(
                            dst[:, j * CK : (j + 1) * CK], lmix[:], AF.Exp
                        )
                return uv, vu

            def c_tile(b, i, uv, vu, e_tiles, last_batch):
                """C = u v^T + v u^T (K=2 matmul), out = E' * C, DMA out."""
                osb = opool.tile([128, N], bf16, tag="o")
                for j in range(NCK):
                    cps = cpool.tile([128, CK], f32, tag="c")
                    nc.tensor.matmul(
                        cps[:],
                        uv[:, i * 128 : (i + 1) * 128],
                        vu[:, j * CK : (j + 1) * CK],
                        start=True,
                        stop=True,
                    )
                    if last_batch and j % 2 == 1:
                        # drain phase: ACT and Pool are idle, DVE is the
                        # bottleneck -- route half the final multiplies
                        # through a PSUM->SBUF copy (DVE 2x) + Pool multiply
                        csb = opool.tile([128, CK], bf16, tag="csb")
                        nc.vector.tensor_copy(csb[:], cps[:])
                        nc.gpsimd.tensor_tensor(
                            osb[:, j * CK : (j + 1) * CK],
                            e_tiles[i][:, j * CK : (j + 1) * CK],
                            csb[:],
                            ALU.mult,
                        )
                    else:
                        nc.vector.tensor_tensor(
                            osb[:, j * CK : (j + 1) * CK],
                            e_tiles[i][:, j * CK : (j + 1) * CK],
                            cps[:],
                            ALU.mult,
                        )
                nc.sync.dma_start(outd[b, i * 128 : (i + 1) * 128, :], osb[:])

            # software pipeline: batch b's stats tiles interleave with batch
            # b-1's output tiles so PE/DVE/DMA trail ACT by one phase
            def emit_pipeline(last_rep):
                prev = None
                for b in range(B_LOC):
                    rowsums = vpool.tile([128, NT], f32, tag="rowsums")
                    r_buf = vpool.tile([128, NT, 2], bf16, tag="rbuf")
                    nc.gpsimd.memset(r_buf[:], 1.0)
                    # cs[0,m] = sum_n r[n] E'[n,m] = w'; cs[1,m] = 2*rowsum[m]
                    cs = cspool.tile([2, N], f32, tag="cs")
                    e_tiles = []
                    for i in range(NT):
                        stats_tile(b, i, zts[b], rowsums, r_buf, cs, e_tiles)
                        if prev is not None:
                            c_tile(prev[0], i, prev[1], prev[2], prev[3], False)
                    uv, vu = batch_tail(b, cs)
                    prev = (b, uv, vu, e_tiles)
                for i in range(NT):
                    c_tile(prev[0], i, prev[1], prev[2], prev[3], last_rep)

            if timing_trip is None:
                for rep in range(repeat):
                    emit_pipeline(rep == repeat - 1)
            else:
                with tc.For_i(0, timing_trip, 1):
                    emit_pipeline(False)
                tiny = singles.tile([2, 2], f32)
                nc.gpsimd.memset(tiny[:], 1.0)
                nc.sync.dma_start(tiny_out[:, :], tiny[:])

    nc.finalize()
    return nc


_NC_CACHE = None


def _get_nc() -> bass.Bass:
    global _NC_CACHE
    if _NC_CACHE is None:
        _NC_CACHE = build_nc()
    return _NC_CACHE


def _make_in_maps(xt: np.ndarray, W: np.ndarray, b: np.ndarray):
    xtT = np.ascontiguousarray(np.asarray(xt).transpose(0, 2, 1)).astype(np.float16)
    Wh = np.ascontiguousarray(np.asarray(W)).astype(np.float16)
    bh = np.ascontiguousarray(np.asarray(b)).reshape(H, 1).astype(np.float32)
    # cst cols: [Ln scale, Ln bias]; cuv cols: Cu(2x2), Cv(2x2)
    # Cu/Cv columns build [ln u; ln v] / [ln v; ln u] from [ln deg; ln(2 rs)]
    # u = exp(-0.5*ldeg - lrs2), v = exp(-0.5*ldeg)
    cst = np.array([[0.5, 1.0], [1.0, 0.0]], dtype=np.float32)
    cuv = np.array(
        [[-0.5, -0.5, -0.5, -0.5], [-1.0, 0.0, 0.0, -1.0]], dtype=np.float32
    )
    return [
        {
            "xtT": xtT[B_LOC * k : B_LOC * (k + 1)],
            "W": Wh,
            "b": bh,
            "cst": cst,
            "cuv": cuv,
        }
        for k in range(N_CORES)
    ]


def run(xt, W, b, trace: bool = False):
    """Run on 8 NeuronCores; returns (out, BassKernelResults)."""
    res = run_bass_kernel_spmd(
        _get_nc(), _make_in_maps(xt, W, b), core_ids=list(range(N_CORES)), trace=trace
    )
    out = np.concatenate(
        [np.asarray(res.results[k]["out"]) for k in range(N_CORES)], axis=0
    )
    return out.astype(np.float32, copy=False), res


def kernel(xt: np.ndarray, W: np.ndarray, b: np.ndarray) -> np.ndarray:
    out, _ = run(xt, W, b, trace=False)
    return out


# revision 79
# speedup vs baseline: 276.4503x; 2.6027x over previous
"""AdaptiveAdjacency Bass kernel for 8 TRN2 NeuronCores.

Reference computation per batch b (N=1024 nodes, H=24 hidden):
    Z   = relu(xt @ W + b)                    (N, H)
    A   = Z @ Z.T                             (N, N)  -- symmetric!
    A   = 0.5*(softmax(A, -1) + softmax(A, -2)) + I
    deg = A.sum(-1);  out = A * deg^-1/2 [row] * deg^-1/2 [col]

Math used here (exploiting symmetry of A_raw):
    E = exp(A_raw - 40)            (shift is softmax-invariant; A_raw max ~54)
    softmax(A,-2) == softmax(A,-1).T, so with r = 1/rowsum(E):
        A_sym[n,m] = E[n,m] * 0.5*(r[n]+r[m]) + I
    Fold "+I" into E:  E' = E + diag(rowsum)  =>  out = E' * C with
        C[n,m] = u[n]v[m] + v[n]u[m],  u = 0.5*r*ds,  v = ds,
        ds = degree^-1/2, degree = 1 + 0.5*colsum(r[n]*E'[n,m])
    C is rank-2 -> one K=2 matmul per output tile; single elementwise
    multiply per output element (the only full-size DVE pass).

Sharding: data-parallel over B=32 across 8 cores (4 batches each);
W/b replicated. Host pre-transposes xt to (B, F, N) fp16 so the tiny
Linear runs as a natural PE matmul (contraction over F on partitions).
"""

import numpy as np

import concourse.bass as bass
import concourse.tile as tile
from concourse import bacc, mybir
from concourse.masks import make_identity
from concourse.bass_utils import run_bass_kernel_spmd

B_FULL = 32
B_LOC = 4  # batches per core
N = 1024
F = 64
H = 24
NT = N // 128  # 8 row tiles
CK = 512  # matmul free chunk (one PSUM bank)
NCK = N // CK
KSHIFT = -40.0  # softmax shift (global constant: softmax-invariant)
N_CORES = 8

f32 = mybir.dt.float32
bf16 = mybir.dt.bfloat16
fp16 = mybir.dt.float16
AF = mybir.ActivationFunctionType
ALU = mybir.AluOpType


_TABLES_PATCHED = False


def _force_single_act_table_set():
    """All activation funcs used here (Exp, Ln, Relu, Copy/Identity) live in
    the natural_log_exp_and_others set. bacc's table-load inserter picks the
    first set containing each function, which thrashes ~2.7us per switch
    between exp_and_others and natural_log. Strip those functions from every
    other set (indices must be preserved) so one table load covers the
    whole kernel."""
    global _TABLES_PATCHED
    if _TABLES_PATCHED:
        return
    _TABLES_PATCHED = True
    import concourse.hw_specs as hw_specs

    orig = hw_specs.get_activation_tables
    keep = {
        AF.Exp,
        AF.Ln,
        AF.Relu,
        AF.Copy,
        AF.Identity,
        AF.Square,
        AF.Abs,
        AF.Sign,
        AF.MemsetZero,
        AF.Is_finite,
    }
    target = "natural_log_exp_and_others"

    def patched(module_arch):
        tables = orig(module_arch)
        if target not in tables:
            return tables
        out = {}
        for name, funcs in tables.items():
            out[name] = funcs if name == target else (funcs - keep)
        return out

    hw_specs.get_activation_tables = patched
    bacc.get_activation_tables = patched


def build_nc(
    repeat: int = 1, timing_trip: int | None = None, ablate: str | None = None
) -> bass.Bass:
    """timing_trip=T builds a timing variant: the whole computation runs in
    an on-device For_i loop T times, writing to internal DRAM scratch with a
    tiny external output, so real device time per iteration can be measured
    by wall-clock differencing of two trip counts (fixed host/transfer costs
    cancel; code size is constant)."""
    _force_single_act_table_set()
    nc = bacc.Bacc()
    xtT = nc.declare_dram_parameter("xtT", [B_LOC, F, N], fp16, isOutput=False)
    Wd = nc.declare_dram_parameter("W", [F, H], fp16, isOutput=False)
    bd = nc.declare_dram_parameter("b", [H, 1], f32, isOutput=False)
    if timing_trip is None:
        outd = nc.declare_dram_parameter("out", [B_LOC, N, N], bf16, isOutput=True)
    else:
        outd = nc.dram_tensor("oscratch", [B_LOC, N, N], bf16)
        tiny_out = nc.declare_dram_parameter("out", [2, 2], f32, isOutput=True)
    # host constants (engine APs must start at partition 0, so these cannot
    # be built with sliced memsets):
    #   cst col 0/1: per-partition scale/bias for the fused colsum Ln
    #   cuv cols 0:2 = Cu, 2:4 = Cv (f32r lhsT for the log-mix matmuls)
    cstd = nc.declare_dram_parameter("cst", [2, 4], f32, isOutput=False)
    cuvd = nc.declare_dram_parameter("cuv", [2, 4], mybir.dt.float32r, isOutput=False)

    with tile.TileContext(nc) as tc:
        with (
            tc.tile_pool(name="singles", bufs=1) as singles,
            tc.tile_pool(name="zpool", bufs=B_LOC) as zpool,
            tc.tile_pool(name="epool", bufs=6 + 2 * NT) as epool,
            tc.tile_pool(name="vpool", bufs=2) as vpool,
            tc.tile_pool(name="opool", bufs=6) as opool,
            tc.tile_pool(name="apool", bufs=2, space="PSUM") as apool,
            tc.tile_pool(name="cpool", bufs=2, space="PSUM") as cpool,
            tc.tile_pool(name="cspool", bufs=1, space="PSUM") as cspool,
        ):
            wsb = singles.tile([F, H], fp16)
            nc.gpsimd.dma_start(wsb[:], Wd[:, :])
            bsb = singles.tile([H, 1], f32)
            nc.gpsimd.dma_start(bsb[:], bd[:, :])
            ident = singles.tile([128, 128], bf16)
            make_identity(nc, ident[:])
            cm40 = singles.tile([128, 1], f32)
            nc.gpsimd.memset(cm40[:], KSHIFT)
            # per-partition [scale, bias] for the fused colsum Ln:
            # row 0: ln(0.5*cs0 + 1.0)   row 1: ln(1.0*cs1 + 0.0)
            cstsb = singles.tile([2, 4], f32)
            nc.gpsimd.dma_start(cstsb[:], cstd[:, :])
            cuvsb = singles.tile([2, 4], mybir.dt.float32r)
            nc.gpsimd.dma_start(cuvsb[:], cuvd[:, :])

            # ---- Z^T = relu(W^T @ xt^T + b) : [H, N] fp16, all batches
            # upfront (fills otherwise-idle engines during pipeline fill and
            # removes the Z chain from the batch-boundary critical path) ----
            zts = []
            for b in range(B_LOC):
                xtsb = zpool.tile([F, N], fp16, tag="xt")
                nc.sync.dma_start(xtsb[:], xtT[b])
                zpsum = apool.tile([H, N], f32, tag="ps")
                for j in range(NCK):
                    nc.tensor.matmul(
                        zpsum[:, j * CK : (j + 1) * CK],
                        wsb[:],
                        xtsb[:, j * CK : (j + 1) * CK],
                        start=True,
                        stop=True,
                    )
                zt = zpool.tile([H, N], fp16, tag="zt")
                # relu on DVE (ACT is the bottleneck engine): (Zpre + b) max 0
                nc.vector.tensor_scalar(
                    zt[:], zpsum[:], bsb[:], 0.0, ALU.add, ALU.max
                )
                # replicas at partitions 32/64/96: A_raw matmuls spread over
                # all four PE row groups (K=24 fits a 32-row group) so up to
                # four streams run concurrently in the array
                zreps = [zt]
                for g in (32, 64, 96):
                    ztg = zpool.tile([g + H, N], fp16, tag=f"zt{g}")
                    nc.gpsimd.dma_start(ztg[g : g + H, :], zt[:])
                    zreps.append(ztg)
                zts.append(zreps)

            def stats_tile(b, i, ztpair, rowsums, r_buf, cs, e_tiles):
                """A_raw matmul, exp(+rowsum), r_i, colsum accumulate.

                The colsum runs on PLAIN E (not E' = E + diag(rowsum)):
                degree = 1.5 + 0.5*colsum(r*E) and cs1 = colsum(E) = rowsum
                (by symmetry), so the diag fix stays off this critical
                chain -- it is emitted later, anywhere before the c-phase.

                The (chunk, tile-parity) pair selects one of the four PE row
                groups via Z replicas at partitions 0/32/64/96, so the two
                chunk matmuls of a tile AND adjacent tiles all overlap in
                the systolic array."""
                zreps = ztpair
                apsum = apool.tile([128, N], f32, tag="ps")
                for j in range(NCK):
                    g = 2 * (i % 2) + j  # 0..3
                    z = zreps[g]
                    base = (32 * g, 32 * g + H)
                    zs = z[base[0] : base[1], :] if g else z[:, :]
                    nc.tensor.matmul(
                        apsum[:, j * CK : (j + 1) * CK],
                        zs[:, i * 128 : (i + 1) * 128],
                        zs[:, j * CK : (j + 1) * CK],
                        start=True,
                        stop=True,
                        tile_position=(32 * g, 0),
                    )
                et = epool.tile([128, N], bf16, tag="E")
                nc.scalar.activation(
                    et[:],
                    apsum[:],
                    AF.Exp,
                    bias=cm40[:],
                    accum_out=rowsums[:, i : i + 1],
                )
                with nc.allow_low_precision("bf16 r for colsum lhsT"):
                    nc.vector.reciprocal(r_buf[:, i, 0:1], rowsums[:, i : i + 1])
                e_tiles.append(et)

            def colsum_tile(i, r_buf, cs, e_tiles):
                """Accumulate cs += [r_i, 1]^T @ E_i. Issued one tile behind
                the exp so the in-order PE never stalls on the exp/recip
                semaphores (measured ~60us of PE stall when issued in-tile)."""
                if ablate == "nocolsum":
                    return
                for j in range(NCK):
                    nc.tensor.matmul(
                        cs[:, j * CK : (j + 1) * CK],
                        r_buf[:, i, :],
                        e_tiles[i][:, j * CK : (j + 1) * CK],
                        start=(i == 0),
                        stop=(i == NT - 1),
                        skip_group_check=True,
                    )

            def diag_fix(i, rowsums, e_tiles):
                """E'[n,n] = E[n,n] + rowsum[n] (folds "+I" into the final
                multiply). Ordered after the colsum reads by Tile's WAR
                tracking; only needed before the c-phase. Runs on the idle
                Pool engine (SBUF-only bf16) to keep DVE free."""
                et = e_tiles[i]
                dtmp = vpool.tile([128, 128], bf16, tag="dtmp")
                nc.gpsimd.tensor_scalar_mul(dtmp[:], ident[:], rowsums[:, i : i + 1])
                nc.gpsimd.tensor_add(
                    et[:, i * 128 : (i + 1) * 128],
                    et[:, i * 128 : (i + 1) * 128],
                    dtmp[:],
                )

            def batch_tail(b, cs):
                """degree -> ds; u, v vectors (free layout).
                degree = 1 + 0.5*cs0 ; v = ds = exp(-0.5*ln(degree))
                u = 0.5*r*ds = exp(-ln(cs1) - 0.5*ln(degree))  (cs1 = 2*rowsum)
                Engine ops must be lane-aligned (partition base 0), so the
                log-domain row mixing runs on the PE (K=2 f32r matmuls
                against a tiny constant lhsT), never across partitions."""
                uv = vpool.tile([2, N], bf16, tag="uv")  # [u; v] (lhsT source)
                vu = vpool.tile([2, N], bf16, tag="vu")  # [v; u] (rhs source)
                lls = vpool.tile([2, N], mybir.dt.float32r, tag="lls")
                nc.scalar.activation(
                    lls[:], cs[:, :], AF.Ln, bias=cstsb[:, 1:2], scale=cstsb[:, 0:1]
                )
                # log-mix matmuls write cpool slots (idle until the c-phase),
                # so the cs accumulator frees right after the Ln and the next
                # batch's colsum can begin during this batch's tail
                for coeff, dst in ((0, uv), (2, vu)):
                    lmix = apool.tile([2, N], f32, tag="ps")
                    for j in range(NCK):
                        nc.tensor.matmul(
                            lmix[:, j * CK : (j + 1) * CK],
                            cuvsb[:, coeff : coeff + 2],
                            lls[:, j * CK : (j + 1) * CK],
                            start=True,
                            stop=True,
                        )
                    # bias ln(0.5) on the u row only (cs1 is rowsum, not
                    # 2*rowsum, so u = exp(mix + ln 0.5))
                    nc.scalar.activation(
                        dst[:],
                        lmix[:],
                        AF.Exp,
                        bias=cstsb[:, 2 + coeff // 2 : 3 + coeff // 2],
                    )
                # partition-64 replicas so odd c-tiles run in PE row group 64
                # (lhsT and rhs must share the same base partition)
                uv64 = vpool.tile([66, N], bf16, tag="uv64")
                nc.gpsimd.dma_start(uv64[64:66, :], uv[:])
                vu64 = vpool.tile([66, N], bf16, tag="vu64")
                nc.gpsimd.dma_start(vu64[64:66, :], vu[:])
                return uv, vu, uv64, vu64

            def c_tile(b, i, uv, vu, uv64, vu64, e_tiles, last_batch):
                """C = u v^T + v u^T (K=2 matmul), out = E' * C, DMA out."""
                osb = opool.tile([128, N], bf16, tag="o")
                if i % 2 == 0:
                    uvs, vus = uv, vu
                else:
                    uvs, vus = uv64[64:66, :], vu64[64:66, :]
                for j in range(NCK):
                    cps = cpool.tile([128, CK], f32, tag="c")
                    nc.tensor.matmul(
                        cps[:],
                        uvs[:, i * 128 : (i + 1) * 128],
                        vus[:, j * CK : (j + 1) * CK],
                        start=True,
                        stop=True,
                    )
                    if last_batch and j % 2 == 1:
                        # drain phase: ACT and Pool are idle, DVE is the
                        # bottleneck -- route half the final multiplies
                        # through a PSUM->SBUF copy (DVE 2x) + Pool multiply
                        csb = opool.tile([128, CK], bf16, tag="csb")
                        nc.vector.tensor_copy(csb[:], cps[:])
                        nc.gpsimd.tensor_tensor(
                            osb[:, j * CK : (j + 1) * CK],
                            e_tiles[i][:, j * CK : (j + 1) * CK],
                            csb[:],
                            ALU.mult,
                        )
                    else:
                        nc.vector.tensor_tensor(
                            osb[:, j * CK : (j + 1) * CK],
                            e_tiles[i][:, j * CK : (j + 1) * CK],
                            cps[:],
                            ALU.mult,
                        )
                if ablate != "nodma":
                    nc.sync.dma_start(outd[b, i * 128 : (i + 1) * 128, :], osb[:])

            # software pipeline: batch b's stats tiles interleave with batch
            # b-1's output tiles so PE/DVE/DMA trail ACT by one phase
            def emit_pipeline(last_rep):
                prev = None
                for b in range(B_LOC):
                    rowsums = vpool.tile([128, NT], f32, tag="rowsums")
                    r_buf = vpool.tile([128, NT, 2], bf16, tag="rbuf")
                    nc.gpsimd.memset(r_buf[:], 1.0)
                    # cs[0,m] = sum_n r[n] E[n,m] = w; cs[1,m] = rowsum[m]
                    cs = cspool.tile([2, N], f32, tag="cs")
                    if ablate == "nocolsum":
                        nc.vector.memset(cs[:], 1.0)
                    e_tiles = []
                    CSLAG = 1
                    for i in range(NT):
                        stats_tile(b, i, zts[b], rowsums, r_buf, cs, e_tiles)
                        if i >= CSLAG:
                            colsum_tile(i - CSLAG, r_buf, cs, e_tiles)
                        if prev is not None and ablate != "statsonly":
                            c_tile(prev[0], i, *prev[1:], False)
                    for i in range(NT - CSLAG, NT):
                        colsum_tile(i, r_buf, cs, e_tiles)
                    for i in range(NT):
                        diag_fix(i, rowsums, e_tiles)
                    uv, vu, uv64, vu64 = batch_tail(b, cs)
                    prev = (b, uv, vu, uv64, vu64, e_tiles)
                if ablate != "statsonly":
                    for i in range(NT):
                        c_tile(prev[0], i, *prev[1:], last_rep)

            if timing_trip is None:
                for rep in range(repeat):
                    emit_pipeline(rep == repeat - 1)
            else:
                with tc.For_i(0, timing_trip, 1):
                    emit_pipeline(False)
                tiny = singles.tile([2, 2], f32)
                nc.gpsimd.memset(tiny[:], 1.0)
                nc.sync.dma_start(tiny_out[:, :], tiny[:])

    nc.finalize()
    return nc


_NC_CACHE = None


def _get_nc() -> bass.Bass:
    global _NC_CACHE
    if _NC_CACHE is None:
        _NC_CACHE = build_nc()
    return _NC_CACHE


def _make_in_maps(xt: np.ndarray, W: np.ndarray, b: np.ndarray):
    xtT = np.ascontiguousarray(np.asarray(xt).transpose(0, 2, 1)).astype(np.float16)
    Wh = np.ascontiguousarray(np.asarray(W)).astype(np.float16)
    bh = np.ascontiguousarray(np.asarray(b)).reshape(H, 1).astype(np.float32)
    # cst cols: [Ln scale, Ln bias, uv-exp bias, vu-exp bias]
    # degree = 1.5 + 0.5*cs0 (plain-E colsum); cs1 = rowsum
    # u = exp(-0.5*ldeg - ln rs + ln 0.5), v = exp(-0.5*ldeg)
    ln_half = float(np.log(0.5))
    cst = np.array(
        [[0.5, 1.5, ln_half, 0.0], [1.0, 0.0, 0.0, ln_half]], dtype=np.float32
    )
    cuv = np.array(
        [[-0.5, -0.5, -0.5, -0.5], [-1.0, 0.0, 0.0, -1.0]], dtype=np.float32
    )
    return [
        {
            "xtT": xtT[B_LOC * k : B_LOC * (k + 1)],
            "W": Wh,
            "b": bh,
            "cst": cst,
            "cuv": cuv,
        }
        for k in range(N_CORES)
    ]


def run(xt, W, b, trace: bool = False):
    """Run on 8 NeuronCores; returns (out, BassKernelResults)."""
    res = run_bass_kernel_spmd(
        _get_nc(), _make_in_maps(xt, W, b), core_ids=list(range(N_CORES)), trace=trace
    )
    out = np.concatenate(
        [np.asarray(res.results[k]["out"]) for k in range(N_CORES)], axis=0
    )
    return out.astype(np.float32, copy=False), res


def kernel(xt: np.ndarray, W: np.ndarray, b: np.ndarray) -> np.ndarray:
    out, _ = run(xt, W, b, trace=False)
    return out
